# revision 28
# baseline (speedup 1.0000x reference)
"""Trainium2 Bass kernel for a pre-norm transformer block (dense_transformer).

Full (unsharded) contract: kernel(**inputs) takes the tensors from
reference.setup_inputs() and returns the full [2, 2048, 1024] output.

Sharding: 8 cores; core c owns batch element b = c//4 and the 512-token
query slice q0 = 512*(c%4) of that batch element.  The host rolls each
core's copy of x[b] by -q0 so that every core's query tokens are rows
0:512 of its input — attention is invariant to key permutation, so K/V
computed from the rolled sequence are exact.  No cross-core collectives:
each core redundantly computes LN1 + K/V for its full batch element
(4 cores share a batch element), then Q/attention/proj/MLP only for its
own 512 tokens.

Layouts on-core (P = 128 partitions):
  ln1T  [128, 8, 2048]  channel-major LN1 output (C on partitions)
  K^T   [128, 2048]     per head-pair (2 heads x 64 dh on partitions)
  Q^T   [128, 512]      per head-pair
  V_g   [128, 16, 520]  token-major V for 8 heads, 65-wide per-head slots
                        with a ones column fused in (col 64) so the AV
                        matmul also yields the softmax denominator
  scores^T [128k, 512q] psum per k-block, exp'd on ScalarE, then
  o~    [65, 512]       psum accumulator over 16 k-blocks (row 64 = l)
  O^T   [128, 8, 512]   normalized attention output, channel-major
  y_tok [128, 4, 1024]  token-major residual stream (after proj)
  ln2T  [128, 8, 512]   channel-major LN2 output
  h1T   [128, 32, 512]  hidden-major GELU(fc1) output
Dense matmuls run as float32r (~fp32 accuracy at full PE rate for free
dim 512); the attention K^T/Q^T/V tiles and exp outputs are bf16.
"""

import sys

for _p in ("/root/.axon_site/_ro/trn_rl_repo", "/opt/trn_rl_repo"):
    if _p not in sys.path:
        sys.path.append(_p)

import numpy as np

import bass_rust
import concourse.bass as bass
import concourse.mybir as mybir
import concourse.tile as tile
from concourse.bass_utils import run_bass_kernel_spmd
from concourse.masks import make_identity
from concourse.vector_clock import ScopedClock

B, N, C = 2, 2048, 1024
H, DH = 16, 64
FF = 4096
NCORES = 8
NQ = 512          # query tokens per core
P = 128
EPS = 1e-5
SCALE = DH ** -0.5
FP32 = mybir.dt.float32
FP32R = mybir.dt.float32r
BF16 = mybir.dt.bfloat16
AF = mybir.ActivationFunctionType
ALU = mybir.AluOpType

NTB = N // P      # 16 token blocks of the full sequence
NCB = C // P      # 8 channel blocks
NQB = NQ // P     # 4 query token blocks
NHB = FF // P     # 32 hidden blocks
SLOT = DH + 1     # 65: V columns per head incl. the fused ones column


class SplitDrainTileContext(tile.TileContext):
    """TileContext whose tail drain carries at most one sem wait per
    instruction — this walrus build rejects >2 sync waits per instruction
    (CoreV3GenImpl setupSyncWait: "Too many sync wait commands")."""

    def _drain_and_barrier(self, tick_clock, wait_clock):
        nc = self.nc
        probe = nc.sync.nop(nofuse=True)
        wait_clock.add_sem_waits(
            probe.ins, ScopedClock({None: tick_clock.global_clock})
        )
        si = probe.ins.sync_info
        waits = list(si.on_wait) if si is not None else []
        updates = list(si.on_update) if si is not None else []
        probe.ins.sync_info = bass_rust.SyncInfo(on_wait=waits[:1], on_update=updates)
        for w in waits[1:]:
            extra = nc.sync.nop(nofuse=True)
            extra.ins.sync_info = bass_rust.SyncInfo(on_wait=[w], on_update=[])
        # Body of TileContext._drain_and_barrier minus add_sem_waits (the
        # waits now live on the nop chain above).
        nc.sync.drain()
        nc.all_engine_barrier()
        assert self.sems is not None
        popped = nc._tile_sem_poison_stack.pop()
        assert popped is self._sem_poison
        nc.clear_and_free_semaphores(list(self.sems.allocated().values()))
        nc.all_engine_barrier()


def _split_waits(nc, maxw=1):
    """Hoist excess sync waits onto same-engine NOPs: this walrus build
    rejects instructions carrying more than `maxw` sync wait commands."""
    snapshots = []
    for f in nc.m.functions:
        for blk in f.blocks:
            snapshots.append((blk, list(blk.instructions)))
    for blk, insts in snapshots:
        rebuilt = []
        for inst in insts:
            si = inst.sync_info
            waits = list(si.on_wait) if si is not None else []
            if len(waits) > maxw:
                for w in waits[:-maxw]:
                    nop = nc.engines[inst.engine].nop(nofuse=True).ins
                    nop.sync_info = bass_rust.SyncInfo(on_wait=[w], on_update=[])
                    rebuilt.append(nop)
                inst.sync_info = bass_rust.SyncInfo(
                    on_wait=waits[-maxw:], on_update=list(si.on_update))
            rebuilt.append(inst)
        blk.instructions = rebuilt


def _layernorm_stats(nc, pool, xt):
    """mean/rstd of xt [128, 1024] over the free axis -> ([128,1], [128,1])."""
    sub = xt.rearrange("p (s f) -> p s f", f=512)
    stats = pool.tile([P, 2, 6], FP32, tag="ln_stats", bufs=4)
    for s in range(2):
        nc.vector.bn_stats(out=stats[:, s, :], in_=sub[:, s, :])
    mv = pool.tile([P, 2], FP32, tag="ln_mv", bufs=4)
    nc.vector.bn_aggr(out=mv[:], in_=stats[:])
    eps = pool.tile([P, 1], FP32, tag="ln_eps", bufs=1)
    nc.vector.memset(eps, EPS)
    rstd = pool.tile([P, 1], FP32, tag="ln_rstd", bufs=4)
    nc.scalar.activation(out=rstd, in_=mv[:, 1:2], func=AF.Sqrt, bias=eps, scale=1.0)
    nc.vector.reciprocal(out=rstd, in_=rstd)
    return mv[:, 0:1], rstd


def build_program():
    nc = bass.Bass("TRN2", target_bir_lowering=False, debug=False)

    x = nc.declare_dram_parameter("x", [N, C], FP32, isOutput=False).ap()
    ln1_g = nc.declare_dram_parameter("ln1_g", [C], FP32, isOutput=False).ap()
    ln1_b = nc.declare_dram_parameter("ln1_b", [C], FP32, isOutput=False).ap()
    qkv_w = nc.declare_dram_parameter("qkv_w", [C, 3 * C], FP32R, isOutput=False).ap()
    proj_w = nc.declare_dram_parameter("proj_w", [C, C], FP32R, isOutput=False).ap()
    proj_b = nc.declare_dram_parameter("proj_b", [C], FP32, isOutput=False).ap()
    ln2_g = nc.declare_dram_parameter("ln2_g", [C], FP32, isOutput=False).ap()
    ln2_b = nc.declare_dram_parameter("ln2_b", [C], FP32, isOutput=False).ap()
    fc1_w = nc.declare_dram_parameter("fc1_w", [C, FF], FP32R, isOutput=False).ap()
    fc1_b = nc.declare_dram_parameter("fc1_b", [FF], FP32, isOutput=False).ap()
    fc2_w = nc.declare_dram_parameter("fc2_w", [FF, C], FP32R, isOutput=False).ap()
    fc2_b = nc.declare_dram_parameter("fc2_b", [C], FP32, isOutput=False).ap()
    out = nc.declare_dram_parameter("out", [NQ, C], FP32, isOutput=True).ap()

    x_t = x.rearrange("(tb p) c -> p tb c", p=P)

    with SplitDrainTileContext(nc) as tc:
        with (
            tc.tile_pool(name="consts", bufs=1) as consts,
            tc.tile_pool(name="stats", bufs=1) as stats_p,
            tc.tile_pool(name="y_pool", bufs=1) as y_pool,
            tc.tile_pool(name="ot_pool", bufs=1) as ot_pool,
            tc.tile_pool(name="psum", bufs=1, space="PSUM") as psum,
        ):
            ident = consts.tile([P, P], FP32)
            make_identity(nc, ident)
            ones32 = consts.tile([P, NTB, 8], FP32)
            nc.vector.memset(ones32, 1.0)
            ones_f = consts.tile([P, DH], FP32)
            nc.vector.memset(ones_f, 1.0)
            ones_col = consts.tile([P, DH], FP32R)
            nc.vector.tensor_copy(out=ones_col, in_=ones_f)

            # per-channel vectors in channel-major [128, NCB] layout
            g1 = consts.tile([P, NCB], FP32)
            b1 = consts.tile([P, NCB], FP32)
            g2 = consts.tile([P, NCB], FP32)
            b2 = consts.tile([P, NCB], FP32)
            pb = consts.tile([P, NCB], FP32)
            fc2b = consts.tile([P, NCB], FP32)
            f1b = consts.tile([P, NHB], FP32)
            for dst, src in ((g1, ln1_g), (b1, ln1_b), (g2, ln2_g), (b2, ln2_b),
                             (pb, proj_b), (fc2b, fc2_b)):
                nc.sync.dma_start(out=dst, in_=src.rearrange("(cb p) -> p cb", p=P))
            nc.sync.dma_start(out=f1b, in_=fc1_b.rearrange("(hb p) -> p hb", p=P))

            y_tok = y_pool.tile([P, NQB, C], FP32)
            O_T = ot_pool.tile([P, NCB, NQ], FP32R)

            # ------------- P0: LN1 + transpose to channel-major -------------
            with tc.tile_pool(name="ln1t_pool", bufs=1) as p_ln1t:
                ln1T = p_ln1t.tile([P, NCB, N], FP32R)
                with tc.tile_pool(name="p0s", bufs=1) as p0s:
                    for tb in range(NTB):
                        xt = p0s.tile([P, C], FP32, tag="x_in", bufs=3)
                        nc.sync.dma_start(out=xt, in_=x_t[:, tb, :])
                        mean, rstd = _layernorm_stats(nc, stats_p, xt)
                        nc.vector.tensor_scalar(
                            out=xt, in0=xt, scalar1=mean, scalar2=rstd,
                            op0=ALU.subtract, op1=ALU.mult,
                        )
                        for cb in range(NCB):
                            pt = psum.tile([P, P], FP32, tag="misc", bufs=2)
                            nc.tensor.transpose(
                                pt, xt[:, cb * P:(cb + 1) * P], ident)
                            nc.scalar.activation(
                                out=ln1T[:, cb, tb * P:(tb + 1) * P], in_=pt,
                                func=AF.Identity, scale=g1[:, cb:cb + 1],
                                bias=b1[:, cb:cb + 1],
                            )

                # ------------- P1-P3: QKV projections + attention -------------
                with tc.tile_pool(name="p1s", bufs=1) as p1s:
                    pending = None

                    def emit_normalize(pair, o_rawA, o_rawB, rl):
                        bca = psum.tile([P, 512], FP32, tag="misc", bufs=2,
                                        name="bca")
                        nc.tensor.matmul(
                            bca[0:DH, :], ones_col[DH:DH + 1, :],
                            rl[DH:DH + 1, 0:512])
                        nc.vector.tensor_mul(out=O_T[0:DH, pair, :],
                                             in0=o_rawA[0:DH, :],
                                             in1=bca[0:DH, :])
                        bcb = psum.tile([P, 512], FP32, tag="misc", bufs=2,
                                        name="bcb")
                        nc.tensor.matmul(
                            bcb[0:DH, :], ones_col[DH:DH + 1, :],
                            rl[DH:DH + 1, 512:1024])
                        # odd head lands on partitions 64:128 of O_T; DVE ops
                        # are partition-aligned, so normalize at base 0 and
                        # move via SBUF->SBUF DMA (partition crossbar)
                        o_sb = p1s.tile([DH, 512], FP32R, tag="o_sb", bufs=2,
                                        name="o_sb")
                        nc.vector.tensor_mul(out=o_sb, in0=o_rawB[0:DH, :],
                                             in1=bcb[0:DH, :])
                        nc.sync.dma_start(out=O_T[DH:P, pair, :], in_=o_sb)

                    for g in range(2):  # head groups of 8 heads
                        V_g = p1s.tile([P, NTB, 8 * SLOT], BF16, tag="V_g", bufs=1)
                        v4 = V_g.rearrange("p t (h s) -> p t h s", s=SLOT)
                        # memset can't produce fp32r on this ISA; cast-copy ones
                        nc.vector.tensor_copy(out=v4[:, :, :, DH:DH + 1],
                                              in_=ones32[:, :, :, None])

                        wv = p1s.tile([P, NCB, 512], FP32R, tag="wv", bufs=1)
                        nc.gpsimd.dma_start(
                            out=wv,
                            in_=qkv_w[:, 2 * C + 512 * g: 2 * C + 512 * (g + 1)]
                            .rearrange("(cb p) n -> p cb n", p=P),
                        )
                        for tb in range(NTB):
                            pv = psum.tile([P, 512], FP32, tag="mm", bufs=2)
                            for cb in range(NCB):
                                nc.tensor.matmul(
                                    pv, ln1T[:, cb, tb * P:(tb + 1) * P],
                                    wv[:, cb, :],
                                    start=(cb == 0), stop=(cb == NCB - 1),
                                )
                            pvh = pv.rearrange("p (h s) -> p h s", s=DH)
                            nc.vector.tensor_copy(out=v4[:, tb, :, 0:DH], in_=pvh)

                        wkg = p1s.tile([P, NCB, 512], FP32R, tag="wkg", bufs=1)
                        nc.gpsimd.dma_start(
                            out=wkg,
                            in_=qkv_w[:, C + 512 * g: C + 512 * (g + 1)]
                            .rearrange("(cb p) n -> p cb n", p=P),
                        )
                        wqg = p1s.tile([P, NCB, 512], FP32R, tag="wqg", bufs=1)
                        nc.gpsimd.dma_start(
                            out=wqg,
                            in_=qkv_w[:, 512 * g: 512 * (g + 1)]
                            .rearrange("(cb p) n -> p cb n", p=P),
                        )
                        for pr in range(4):  # head pairs within the group
                            pair = 4 * g + pr
                            wk = wkg[:, :, pr * P:(pr + 1) * P]
                            wq = wqg[:, :, pr * P:(pr + 1) * P]
                            KT = p1s.tile([P, N], BF16, tag="KT", bufs=2)
                            for tb in range(4):  # 512-token blocks
                                pk = psum.tile([P, 512], FP32, tag="mm", bufs=2)
                                for cb in range(NCB):
                                    nc.tensor.matmul(
                                        pk, wk[:, cb, :],
                                        ln1T[:, cb, tb * 512:(tb + 1) * 512],
                                        start=(cb == 0), stop=(cb == NCB - 1),
                                    )
                                nc.vector.tensor_copy(
                                    out=KT[:, tb * 512:(tb + 1) * 512], in_=pk)
                            QT = p1s.tile([P, NQ], BF16, tag="QT", bufs=2)
                            pq = psum.tile([P, 512], FP32, tag="mm", bufs=2)
                            for cb in range(NCB):
                                nc.tensor.matmul(
                                    pq, wq[:, cb, :], ln1T[:, cb, 0:NQ],
                                    start=(cb == 0), stop=(cb == NCB - 1),
                                )
                            nc.vector.tensor_copy(out=QT, in_=pq)

                            # flash attention over 16 key blocks; the two heads
                            # of the pair run as packed K=64 row-tiles
                            oa = psum.tile([P, 512], FP32, tag="acc", bufs=2)
                            ob_ = psum.tile([P, 512], FP32, tag="acc", bufs=2)
                            sl_a = slice(2 * pr * SLOT, (2 * pr) * SLOT + SLOT)
                            sl_b = slice((2 * pr + 1) * SLOT, (2 * pr + 2) * SLOT)
                            def emit_av(k2, ea, eb):
                                for j in range(2):
                                    kb = 2 * k2 + j
                                    nc.tensor.matmul(
                                        oa[0:SLOT, :], V_g[:, kb, sl_a],
                                        ea[:, j, :],
                                        start=(kb == 0), stop=(kb == NTB - 1),
                                    )
                                    nc.tensor.matmul(
                                        ob_[0:SLOT, :], V_g[:, kb, sl_b],
                                        eb[:, j, :],
                                        start=(kb == 0), stop=(kb == NTB - 1),
                                    )

                            av_pending = None
                            for k2 in range(NTB // 2):
                                sa = psum.tile([P, 2, 512], FP32, tag="mm", bufs=2)
                                sb = psum.tile([P, 2, 512], FP32, tag="mm", bufs=2)
                                for j in range(2):
                                    kb = 2 * k2 + j
                                    ks = slice(kb * P, (kb + 1) * P)
                                    nc.tensor.matmul(
                                        sa[:, j, :], KT[0:DH, ks], QT[0:DH, :],
                                        tile_position=(0, 0),
                                    )
                                    nc.tensor.matmul(
                                        sb[:, j, :], KT[DH:P, ks], QT[DH:P, :],
                                        tile_position=(DH, 0),
                                    )
                                ea = p1s.tile([P, 2, 512], BF16, tag="ea", bufs=3)
                                nc.scalar.activation(out=ea, in_=sa, func=AF.Exp,
                                                     scale=SCALE)
                                eb = p1s.tile([P, 2, 512], BF16, tag="eb", bufs=3)
                                nc.scalar.activation(out=eb, in_=sb, func=AF.Exp,
                                                     scale=SCALE)
                                if av_pending is not None:
                                    emit_av(*av_pending)
                                av_pending = (k2, ea, eb)
                            emit_av(*av_pending)
                            # evacuate o~ + l to SBUF right away (frees the
                            # PSUM accumulators for the next pair), 1/l on DVE,
                            # and defer broadcast+scale one pair so the PE
                            # never stalls on the reciprocal
                            o_rawA = p1s.tile([SLOT, 512], FP32, tag="o_rawA",
                                              bufs=2)
                            nc.vector.tensor_copy(out=o_rawA, in_=oa[0:SLOT, :])
                            o_rawB = p1s.tile([SLOT, 512], FP32, tag="o_rawB",
                                              bufs=2)
                            nc.vector.tensor_copy(out=o_rawB, in_=ob_[0:SLOT, :])
                            rl = stats_p.tile([P, 2 * 512], FP32R, tag="rl",
                                              bufs=2)
                            with nc.allow_low_precision(
                                    reason="softmax 1/l feeds an fp32r matmul"):
                                nc.vector.reciprocal(out=rl[DH:DH + 1, 0:512],
                                                     in_=o_rawA[DH:DH + 1, :])
                                nc.vector.reciprocal(out=rl[DH:DH + 1, 512:1024],
                                                     in_=o_rawB[DH:DH + 1, :])
                            if pending is not None:
                                emit_normalize(*pending)
                            pending = (pair, o_rawA, o_rawB, rl)

                    if pending is not None:
                        emit_normalize(*pending)
                        pending = None

            # ------------- P4: proj + residual -> y_tok -------------
            # swapped operands: lhsT = O^T (channel-major), rhs = natural
            # proj_w rows -> psum is token-major y directly (no transposes)
            with tc.tile_pool(name="p4s", bufs=1) as p4s:
                x_tok = p4s.tile([P, NQB, C], FP32, tag="x_res", bufs=1)
                nc.sync.dma_start(out=x_tok, in_=x_t[:, 0:NQB, :])
                pb_bc = bass.AP(tensor=proj_b.tensor, offset=proj_b.offset,
                                ap=[[0, P], [1, C]])
                pbt = p4s.tile([P, C], FP32, tag="pbt", bufs=1)
                nc.sync.dma_start(out=pbt, in_=pb_bc)
                for ts in range(NQB):
                    nc.vector.tensor_add(out=x_tok[:, ts, :],
                                         in0=x_tok[:, ts, :], in1=pbt)
                wpf = p4s.tile([P, NCB, C], FP32R, tag="wpf", bufs=1)
                nc.gpsimd.dma_start(
                    out=wpf, in_=proj_w.rearrange("(cb p) n -> p cb n", p=P))
                for ts in range(NQB):
                    for ocb in range(2):
                        py = psum.tile([P, 512], FP32, tag="mm", bufs=2)
                        for cb in range(NCB):
                            nc.tensor.matmul(
                                py, O_T[:, cb, ts * P:(ts + 1) * P],
                                wpf[:, cb, ocb * 512:(ocb + 1) * 512],
                                start=(cb == 0), stop=(cb == NCB - 1))
                        nc.vector.tensor_add(
                            out=y_tok[:, ts, ocb * 512:(ocb + 1) * 512],
                            in0=py,
                            in1=x_tok[:, ts, ocb * 512:(ocb + 1) * 512])

            # ------------- P5: LN2 -> ln2T (channel-major) -------------
            with tc.tile_pool(name="ln2t_pool", bufs=1) as p_ln2t:
                ln2T = p_ln2t.tile([P, NCB, NQ], FP32R)
                with tc.tile_pool(name="p5s", bufs=1) as p5s:
                    ycts = []
                    for ts in range(NQB):
                        yc_t = p5s.tile([P, C], FP32, tag=f"yc_t{ts}", bufs=1,
                                        name=f"yc_t{ts}")
                        mean, rstd = _layernorm_stats(nc, stats_p, y_tok[:, ts, :])
                        nc.vector.tensor_scalar(
                            out=yc_t, in0=y_tok[:, ts, :], scalar1=mean,
                            scalar2=rstd, op0=ALU.subtract, op1=ALU.mult,
                        )
                        ycts.append(yc_t)
                    # cb-outer so ln2T[:, cb, :] completes per channel block
                    # and fc1's accumulation can begin after the first one
                    for cb in range(NCB):
                        for ts in range(NQB):
                            pt = psum.tile([P, P], FP32, tag="misc", bufs=2)
                            nc.tensor.transpose(
                                pt, ycts[ts][:, cb * P:(cb + 1) * P], ident)
                            nc.scalar.activation(
                                out=ln2T[:, cb, ts * P:(ts + 1) * P], in_=pt,
                                func=AF.Identity, scale=g2[:, cb:cb + 1],
                                bias=b2[:, cb:cb + 1],
                            )

                # ------------- P6: fc1 + GELU -> h1T -------------
                with tc.tile_pool(name="h1_pool", bufs=1) as p_h1:
                    h1T = p_h1.tile([P, NHB, NQ], FP32R)
                    with tc.tile_pool(name="p6s", bufs=1) as p6s:
                        for hc in range(4):  # 8-hb chunks of fc1_w
                            w1 = p6s.tile([P, NCB, 8 * P], FP32R, tag="w1",
                                          bufs=2)
                            nc.gpsimd.dma_start(
                                out=w1,
                                in_=fc1_w[:, hc * 8 * P:(hc + 1) * 8 * P]
                                .rearrange("(cb p) n -> p cb n", p=P),
                            )
                            for hl in range(8):
                                hb = hc * 8 + hl
                                ph = psum.tile([P, 512], FP32, tag="mm", bufs=2)
                                for cb in range(NCB):
                                    nc.tensor.matmul(
                                        ph, w1[:, cb, hl * P:(hl + 1) * P],
                                        ln2T[:, cb, :],
                                        start=(cb == 0), stop=(cb == NCB - 1))
                                nc.scalar.activation(
                                    out=h1T[:, hb, :], in_=ph, func=AF.Gelu,
                                    bias=f1b[:, hb:hb + 1], scale=1.0)

                    # ------------- P7: fc2 + residual -> out -------------
                    # swapped operands: lhsT = h1T (hidden-major), rhs =
                    # natural fc2_w rows -> token-major out, no transposes.
                    # 8 psum accumulators (4 ts x 2 ocb) live across the 4
                    # hb-chunks, overlapping fc1 production order.
                    with tc.tile_pool(name="p7s", bufs=1) as p7s:
                        ob_bc = bass.AP(tensor=fc2_b.tensor, offset=fc2_b.offset,
                                        ap=[[0, P], [1, C]])
                        obt = p7s.tile([P, C], FP32, tag="obt", bufs=1)
                        nc.sync.dma_start(out=obt, in_=ob_bc)
                        for ts in range(NQB):
                            nc.vector.tensor_add(out=y_tok[:, ts, :],
                                                 in0=y_tok[:, ts, :], in1=obt)
                        out_tok = p7s.tile([P, NQB, C], FP32, tag="out_tok",
                                           bufs=1)
                        pos = [psum.tile([P, 2, 512], FP32, tag="mm", bufs=2,
                                         name=f"po_mm{i}") for i in range(2)]
                        poa = [psum.tile([P, 512], FP32, tag="acc", bufs=2,
                                         name=f"po_acc{i}") for i in range(2)]
                        pom = [psum.tile([P, 512], FP32, tag="misc", bufs=2,
                                         name=f"po_misc{i}") for i in range(2)]
                        po = {(0, 0): pos[0][:, 0, :], (0, 1): pos[0][:, 1, :],
                              (1, 0): pos[1][:, 0, :], (1, 1): pos[1][:, 1, :],
                              (2, 0): poa[0], (2, 1): poa[1],
                              (3, 0): pom[0], (3, 1): pom[1]}
                        for hc in range(4):
                            w2 = p7s.tile([P, 8, C], FP32R, tag="w2", bufs=2)
                            nc.gpsimd.dma_start(
                                out=w2,
                                in_=fc2_w[hc * 8 * P:(hc + 1) * 8 * P, :]
                                .rearrange("(hb p) n -> p hb n", p=P),
                            )
                            for hl in range(8):
                                hb = hc * 8 + hl
                                for ts in range(NQB):
                                    for ocb in range(2):
                                        nc.tensor.matmul(
                                            po[(ts, ocb)],
                                            h1T[:, hb, ts * P:(ts + 1) * P],
                                            w2[:, hl, ocb * 512:(ocb + 1) * 512],
                                            start=(hb == 0), stop=(hb == NHB - 1))
                        for ts in range(NQB):
                            for ocb in range(2):
                                nc.vector.tensor_add(
                                    out=out_tok[:, ts, ocb * 512:(ocb + 1) * 512],
                                    in0=po[(ts, ocb)],
                                    in1=y_tok[:, ts, ocb * 512:(ocb + 1) * 512])
                        nc.sync.dma_start(
                            out=out.rearrange("(tb p) c -> p tb c", p=P),
                            in_=out_tok)

    _split_waits(nc)
    return nc


_NC_CACHE = None


def kernel(**inputs):
    global _NC_CACHE
    if _NC_CACHE is None:
        _NC_CACHE = build_program()
    nc = _NC_CACHE

    x = np.ascontiguousarray(np.asarray(inputs["x"], dtype=np.float32))
    weights = {
        k: np.ascontiguousarray(np.asarray(inputs[k], dtype=np.float32))
        for k in ("ln1_g", "ln1_b", "qkv_w", "proj_w", "proj_b",
                  "ln2_g", "ln2_b", "fc1_w", "fc1_b", "fc2_w", "fc2_b")
    }
    in_maps = []
    for c in range(NCORES):
        b, q0 = c // 4, NQ * (c % 4)
        xb = np.ascontiguousarray(np.roll(x[b], -q0, axis=0))
        in_maps.append({"x": xb, **weights})

    res = run_bass_kernel_spmd(nc, in_maps, list(range(NCORES)))
    out = np.empty((B, N, C), dtype=np.float32)
    for c in range(NCORES):
        b, q0 = c // 4, NQ * (c % 4)
        out[b, q0:q0 + NQ] = res.results[c]["out"]
    return out


# revision 29
# speedup vs baseline: 1.0689x; 1.0689x over previous
"""Trainium2 Bass kernel for a pre-norm transformer block (dense_transformer).

Full (unsharded) contract: kernel(**inputs) takes the tensors from
reference.setup_inputs() and returns the full [2, 2048, 1024] output.

Sharding: 8 cores; core c owns batch element b = c//4 and the 512-token
query slice q0 = 512*(c%4) of that batch element.  The host rolls each
core's copy of x[b] by -q0 so that every core's query tokens are rows
0:512 of its input — attention is invariant to key permutation, so K/V
computed from the rolled sequence are exact.  No cross-core collectives:
each core redundantly computes LN1 + K/V for its full batch element
(4 cores share a batch element), then Q/attention/proj/MLP only for its
own 512 tokens.

Layouts on-core (P = 128 partitions):
  ln1T  [128, 8, 2048]  channel-major LN1 output (C on partitions)
  K^T   [128, 2048]     per head-pair (2 heads x 64 dh on partitions)
  Q^T   [128, 512]      per head-pair
  V_g   [128, 16, 520]  token-major V for 8 heads, 65-wide per-head slots
                        with a ones column fused in (col 64) so the AV
                        matmul also yields the softmax denominator
  scores^T [128k, 512q] psum per k-block, exp'd on ScalarE, then
  o~    [65, 512]       psum accumulator over 16 k-blocks (row 64 = l)
  O^T   [128, 8, 512]   normalized attention output, channel-major
  y_tok [128, 4, 1024]  token-major residual stream (after proj)
  ln2T  [128, 8, 512]   channel-major LN2 output
  h1T   [128, 32, 512]  hidden-major GELU(fc1) output
Dense matmuls run as float32r (~fp32 accuracy at full PE rate for free
dim 512); the attention K^T/Q^T/V tiles and exp outputs are bf16.
"""

import sys

for _p in ("/root/.axon_site/_ro/trn_rl_repo", "/opt/trn_rl_repo"):
    if _p not in sys.path:
        sys.path.append(_p)

import numpy as np

import bass_rust
import concourse.bass as bass
import concourse.mybir as mybir
import concourse.tile as tile
from concourse.bass_utils import run_bass_kernel_spmd
from concourse.masks import make_identity
from concourse.vector_clock import ScopedClock

B, N, C = 2, 2048, 1024
H, DH = 16, 64
FF = 4096
NCORES = 8
NQ = 512          # query tokens per core
P = 128
EPS = 1e-5
SCALE = DH ** -0.5
FP32 = mybir.dt.float32
FP32R = mybir.dt.float32r
BF16 = mybir.dt.bfloat16
AF = mybir.ActivationFunctionType
ALU = mybir.AluOpType

NTB = N // P      # 16 token blocks of the full sequence
NCB = C // P      # 8 channel blocks
NQB = NQ // P     # 4 query token blocks
NHB = FF // P     # 32 hidden blocks
SLOT = DH + 1     # 65: V columns per head incl. the fused ones column


class SplitDrainTileContext(tile.TileContext):
    """TileContext whose tail drain carries at most one sem wait per
    instruction — this walrus build rejects >2 sync waits per instruction
    (CoreV3GenImpl setupSyncWait: "Too many sync wait commands")."""

    def _drain_and_barrier(self, tick_clock, wait_clock):
        nc = self.nc
        probe = nc.sync.nop(nofuse=True)
        wait_clock.add_sem_waits(
            probe.ins, ScopedClock({None: tick_clock.global_clock})
        )
        si = probe.ins.sync_info
        waits = list(si.on_wait) if si is not None else []
        updates = list(si.on_update) if si is not None else []
        probe.ins.sync_info = bass_rust.SyncInfo(on_wait=waits[:1], on_update=updates)
        for w in waits[1:]:
            extra = nc.sync.nop(nofuse=True)
            extra.ins.sync_info = bass_rust.SyncInfo(on_wait=[w], on_update=[])
        # Body of TileContext._drain_and_barrier minus add_sem_waits (the
        # waits now live on the nop chain above).
        nc.sync.drain()
        nc.all_engine_barrier()
        assert self.sems is not None
        popped = nc._tile_sem_poison_stack.pop()
        assert popped is self._sem_poison
        nc.clear_and_free_semaphores(list(self.sems.allocated().values()))
        nc.all_engine_barrier()


def _split_waits(nc, maxw=1):
    """Hoist excess sync waits onto same-engine NOPs: this walrus build
    rejects instructions carrying more than `maxw` sync wait commands."""
    snapshots = []
    for f in nc.m.functions:
        for blk in f.blocks:
            snapshots.append((blk, list(blk.instructions)))
    for blk, insts in snapshots:
        rebuilt = []
        for inst in insts:
            si = inst.sync_info
            waits = list(si.on_wait) if si is not None else []
            if len(waits) > maxw:
                for w in waits[:-maxw]:
                    nop = nc.engines[inst.engine].nop(nofuse=True).ins
                    nop.sync_info = bass_rust.SyncInfo(on_wait=[w], on_update=[])
                    rebuilt.append(nop)
                inst.sync_info = bass_rust.SyncInfo(
                    on_wait=waits[-maxw:], on_update=list(si.on_update))
            rebuilt.append(inst)
        blk.instructions = rebuilt


def _layernorm_stats(nc, pool, xt):
    """mean/rstd of xt [128, 1024] over the free axis -> ([128,1], [128,1])."""
    sub = xt.rearrange("p (s f) -> p s f", f=512)
    stats = pool.tile([P, 2, 6], FP32, tag="ln_stats", bufs=4)
    for s in range(2):
        nc.vector.bn_stats(out=stats[:, s, :], in_=sub[:, s, :])
    mv = pool.tile([P, 2], FP32, tag="ln_mv", bufs=4)
    nc.vector.bn_aggr(out=mv[:], in_=stats[:])
    eps = pool.tile([P, 1], FP32, tag="ln_eps", bufs=1)
    nc.vector.memset(eps, EPS)
    rstd = pool.tile([P, 1], FP32, tag="ln_rstd", bufs=4)
    nc.scalar.activation(out=rstd, in_=mv[:, 1:2], func=AF.Sqrt, bias=eps, scale=1.0)
    nc.vector.reciprocal(out=rstd, in_=rstd)
    return mv[:, 0:1], rstd


def build_program():
    nc = bass.Bass("TRN2", target_bir_lowering=False, debug=False)

    x = nc.declare_dram_parameter("x", [N, C], FP32, isOutput=False).ap()
    ln1_g = nc.declare_dram_parameter("ln1_g", [C], FP32, isOutput=False).ap()
    ln1_b = nc.declare_dram_parameter("ln1_b", [C], FP32, isOutput=False).ap()
    qkv_w = nc.declare_dram_parameter("qkv_w", [C, 3 * C], FP32R, isOutput=False).ap()
    proj_w = nc.declare_dram_parameter("proj_w", [C, C], FP32R, isOutput=False).ap()
    proj_b = nc.declare_dram_parameter("proj_b", [C], FP32, isOutput=False).ap()
    ln2_g = nc.declare_dram_parameter("ln2_g", [C], FP32, isOutput=False).ap()
    ln2_b = nc.declare_dram_parameter("ln2_b", [C], FP32, isOutput=False).ap()
    fc1_w = nc.declare_dram_parameter("fc1_w", [C, FF], FP32R, isOutput=False).ap()
    fc1_b = nc.declare_dram_parameter("fc1_b", [FF], FP32, isOutput=False).ap()
    fc2_w = nc.declare_dram_parameter("fc2_w", [FF, C], FP32R, isOutput=False).ap()
    fc2_b = nc.declare_dram_parameter("fc2_b", [C], FP32, isOutput=False).ap()
    out = nc.declare_dram_parameter("out", [NQ, C], FP32, isOutput=True).ap()

    x_t = x.rearrange("(tb p) c -> p tb c", p=P)

    with SplitDrainTileContext(nc) as tc:
        with (
            tc.tile_pool(name="consts", bufs=1) as consts,
            tc.tile_pool(name="stats", bufs=1) as stats_p,
            tc.tile_pool(name="y_pool", bufs=1) as y_pool,
            tc.tile_pool(name="ot_pool", bufs=1) as ot_pool,
            tc.tile_pool(name="psum", bufs=1, space="PSUM") as psum,
        ):
            ident = consts.tile([P, P], FP32)
            make_identity(nc, ident)
            ones32 = consts.tile([P, NTB, 8], FP32)
            nc.vector.memset(ones32, 1.0)
            ones_f = consts.tile([P, DH], FP32)
            nc.vector.memset(ones_f, 1.0)
            ones_col = consts.tile([P, DH], FP32R)
            nc.vector.tensor_copy(out=ones_col, in_=ones_f)

            # per-channel vectors in channel-major [128, NCB] layout
            g1 = consts.tile([P, NCB], FP32)
            b1 = consts.tile([P, NCB], FP32)
            g2 = consts.tile([P, NCB], FP32)
            b2 = consts.tile([P, NCB], FP32)
            pb = consts.tile([P, NCB], FP32)
            fc2b = consts.tile([P, NCB], FP32)
            f1b = consts.tile([P, NHB], FP32)
            for dst, src in ((g1, ln1_g), (b1, ln1_b), (g2, ln2_g), (b2, ln2_b),
                             (pb, proj_b), (fc2b, fc2_b)):
                nc.sync.dma_start(out=dst, in_=src.rearrange("(cb p) -> p cb", p=P))
            nc.sync.dma_start(out=f1b, in_=fc1_b.rearrange("(hb p) -> p hb", p=P))

            y_tok = y_pool.tile([P, NQB, C], FP32)
            O_T = ot_pool.tile([P, NCB, NQ], FP32R)

            # ------------- P0: LN1 + transpose to channel-major -------------
            with tc.tile_pool(name="ln1t_pool", bufs=1) as p_ln1t:
                ln1T = p_ln1t.tile([P, NCB, N], FP32R)
                with tc.tile_pool(name="p0s", bufs=1) as p0s:
                    for tb in range(NTB):
                        xt = p0s.tile([P, C], FP32, tag="x_in", bufs=3)
                        nc.sync.dma_start(out=xt, in_=x_t[:, tb, :])
                        mean, rstd = _layernorm_stats(nc, stats_p, xt)
                        nc.vector.tensor_scalar(
                            out=xt, in0=xt, scalar1=mean, scalar2=rstd,
                            op0=ALU.subtract, op1=ALU.mult,
                        )
                        for cb in range(NCB):
                            pt = psum.tile([P, P], FP32, tag="misc", bufs=2)
                            nc.tensor.transpose(
                                pt, xt[:, cb * P:(cb + 1) * P], ident)
                            nc.scalar.activation(
                                out=ln1T[:, cb, tb * P:(tb + 1) * P], in_=pt,
                                func=AF.Identity, scale=g1[:, cb:cb + 1],
                                bias=b1[:, cb:cb + 1],
                            )

                # ------------- P1-P3: QKV projections + attention -------------
                with tc.tile_pool(name="p1s", bufs=1) as p1s:
                    pending = None

                    def emit_normalize(pair, o_rawA, o_rawB, rl):
                        bca = psum.tile([P, 512], FP32, tag="misc", bufs=2,
                                        name="bca")
                        nc.tensor.matmul(
                            bca[0:DH, :], ones_col[DH:DH + 1, :],
                            rl[DH:DH + 1, 0:512])
                        nc.vector.tensor_mul(out=O_T[0:DH, pair, :],
                                             in0=o_rawA[0:DH, :],
                                             in1=bca[0:DH, :])
                        bcb = psum.tile([P, 512], FP32, tag="misc", bufs=2,
                                        name="bcb")
                        nc.tensor.matmul(
                            bcb[0:DH, :], ones_col[DH:DH + 1, :],
                            rl[DH:DH + 1, 512:1024])
                        # odd head lands on partitions 64:128 of O_T; DVE ops
                        # are partition-aligned, so normalize at base 0 and
                        # move via SBUF->SBUF DMA (partition crossbar)
                        o_sb = p1s.tile([DH, 512], FP32R, tag="o_sb", bufs=2,
                                        name="o_sb")
                        nc.vector.tensor_mul(out=o_sb, in0=o_rawB[0:DH, :],
                                             in1=bcb[0:DH, :])
                        nc.sync.dma_start(out=O_T[DH:P, pair, :], in_=o_sb)

                    for g in range(2):  # head groups of 8 heads
                        V_g = p1s.tile([P, NTB, 8 * SLOT], BF16, tag="V_g", bufs=1)
                        v4 = V_g.rearrange("p t (h s) -> p t h s", s=SLOT)
                        # memset can't produce fp32r on this ISA; cast-copy ones
                        nc.vector.tensor_copy(out=v4[:, :, :, DH:DH + 1],
                                              in_=ones32[:, :, :, None])

                        wv = p1s.tile([P, NCB, 512], FP32R, tag="wv", bufs=1)
                        nc.scalar.dma_start(
                            out=wv,
                            in_=qkv_w[:, 2 * C + 512 * g: 2 * C + 512 * (g + 1)]
                            .rearrange("(cb p) n -> p cb n", p=P),
                        )
                        for tb in range(NTB):
                            pv = psum.tile([P, 512], FP32, tag="mm", bufs=2)
                            for cb in range(NCB):
                                nc.tensor.matmul(
                                    pv, ln1T[:, cb, tb * P:(tb + 1) * P],
                                    wv[:, cb, :],
                                    start=(cb == 0), stop=(cb == NCB - 1),
                                )
                            pvh = pv.rearrange("p (h s) -> p h s", s=DH)
                            nc.vector.tensor_copy(out=v4[:, tb, :, 0:DH], in_=pvh)

                        wkg = p1s.tile([P, NCB, 512], FP32R, tag="wkg", bufs=1)
                        nc.scalar.dma_start(
                            out=wkg,
                            in_=qkv_w[:, C + 512 * g: C + 512 * (g + 1)]
                            .rearrange("(cb p) n -> p cb n", p=P),
                        )
                        wqg = p1s.tile([P, NCB, 512], FP32R, tag="wqg", bufs=1)
                        nc.scalar.dma_start(
                            out=wqg,
                            in_=qkv_w[:, 512 * g: 512 * (g + 1)]
                            .rearrange("(cb p) n -> p cb n", p=P),
                        )
                        for pr in range(4):  # head pairs within the group
                            pair = 4 * g + pr
                            wk = wkg[:, :, pr * P:(pr + 1) * P]
                            wq = wqg[:, :, pr * P:(pr + 1) * P]
                            KT = p1s.tile([P, N], BF16, tag="KT", bufs=2)
                            for tb in range(4):  # 512-token blocks
                                pk = psum.tile([P, 512], FP32, tag="mm", bufs=2)
                                for cb in range(NCB):
                                    nc.tensor.matmul(
                                        pk, wk[:, cb, :],
                                        ln1T[:, cb, tb * 512:(tb + 1) * 512],
                                        start=(cb == 0), stop=(cb == NCB - 1),
                                    )
                                nc.vector.tensor_copy(
                                    out=KT[:, tb * 512:(tb + 1) * 512], in_=pk)
                            QT = p1s.tile([P, NQ], BF16, tag="QT", bufs=2)
                            pq = psum.tile([P, 512], FP32, tag="mm", bufs=2)
                            for cb in range(NCB):
                                nc.tensor.matmul(
                                    pq, wq[:, cb, :], ln1T[:, cb, 0:NQ],
                                    start=(cb == 0), stop=(cb == NCB - 1),
                                )
                            nc.vector.tensor_copy(out=QT, in_=pq)

                            # flash attention over 16 key blocks; the two heads
                            # of the pair run as packed K=64 row-tiles
                            oa = psum.tile([P, 512], FP32, tag="acc", bufs=2)
                            ob_ = psum.tile([P, 512], FP32, tag="acc", bufs=2)
                            sl_a = slice(2 * pr * SLOT, (2 * pr) * SLOT + SLOT)
                            sl_b = slice((2 * pr + 1) * SLOT, (2 * pr + 2) * SLOT)
                            def emit_av(k2, ea, eb):
                                for j in range(2):
                                    kb = 2 * k2 + j
                                    nc.tensor.matmul(
                                        oa[0:SLOT, :], V_g[:, kb, sl_a],
                                        ea[:, j, :],
                                        start=(kb == 0), stop=(kb == NTB - 1),
                                    )
                                    nc.tensor.matmul(
                                        ob_[0:SLOT, :], V_g[:, kb, sl_b],
                                        eb[:, j, :],
                                        start=(kb == 0), stop=(kb == NTB - 1),
                                    )

                            av_pending = None
                            for k2 in range(NTB // 2):
                                sa = psum.tile([P, 2, 512], FP32, tag="mm", bufs=2)
                                sb = psum.tile([P, 2, 512], FP32, tag="mm", bufs=2)
                                for j in range(2):
                                    kb = 2 * k2 + j
                                    ks = slice(kb * P, (kb + 1) * P)
                                    nc.tensor.matmul(
                                        sa[:, j, :], KT[0:DH, ks], QT[0:DH, :],
                                        tile_position=(0, 0),
                                    )
                                    nc.tensor.matmul(
                                        sb[:, j, :], KT[DH:P, ks], QT[DH:P, :],
                                        tile_position=(DH, 0),
                                    )
                                ea = p1s.tile([P, 2, 512], BF16, tag="ea", bufs=3)
                                nc.scalar.activation(out=ea, in_=sa, func=AF.Exp,
                                                     scale=SCALE)
                                eb = p1s.tile([P, 2, 512], BF16, tag="eb", bufs=3)
                                nc.scalar.activation(out=eb, in_=sb, func=AF.Exp,
                                                     scale=SCALE)
                                if av_pending is not None:
                                    emit_av(*av_pending)
                                av_pending = (k2, ea, eb)
                            emit_av(*av_pending)
                            # evacuate o~ + l to SBUF right away (frees the
                            # PSUM accumulators for the next pair), 1/l on DVE,
                            # and defer broadcast+scale one pair so the PE
                            # never stalls on the reciprocal
                            o_rawA = p1s.tile([SLOT, 512], FP32, tag="o_rawA",
                                              bufs=2)
                            nc.vector.tensor_copy(out=o_rawA, in_=oa[0:SLOT, :])
                            o_rawB = p1s.tile([SLOT, 512], FP32, tag="o_rawB",
                                              bufs=2)
                            nc.vector.tensor_copy(out=o_rawB, in_=ob_[0:SLOT, :])
                            rl = stats_p.tile([P, 2 * 512], FP32R, tag="rl",
                                              bufs=2)
                            with nc.allow_low_precision(
                                    reason="softmax 1/l feeds an fp32r matmul"):
                                nc.vector.reciprocal(out=rl[DH:DH + 1, 0:512],
                                                     in_=o_rawA[DH:DH + 1, :])
                                nc.vector.reciprocal(out=rl[DH:DH + 1, 512:1024],
                                                     in_=o_rawB[DH:DH + 1, :])
                            if pending is not None:
                                emit_normalize(*pending)
                            pending = (pair, o_rawA, o_rawB, rl)

                    if pending is not None:
                        emit_normalize(*pending)
                        pending = None

            # ------------- P4: proj + residual -> y_tok -------------
            # swapped operands: lhsT = O^T (channel-major), rhs = natural
            # proj_w rows -> psum is token-major y directly (no transposes)
            with tc.tile_pool(name="p4s", bufs=1) as p4s:
                x_tok = p4s.tile([P, NQB, C], FP32, tag="x_res", bufs=1)
                nc.sync.dma_start(out=x_tok, in_=x_t[:, 0:NQB, :])
                pb_bc = bass.AP(tensor=proj_b.tensor, offset=proj_b.offset,
                                ap=[[0, P], [1, C]])
                pbt = p4s.tile([P, C], FP32, tag="pbt", bufs=1)
                nc.sync.dma_start(out=pbt, in_=pb_bc)
                for ts in range(NQB):
                    nc.vector.tensor_add(out=x_tok[:, ts, :],
                                         in0=x_tok[:, ts, :], in1=pbt)
                wpf = p4s.tile([P, NCB, C], FP32R, tag="wpf", bufs=1)
                nc.scalar.dma_start(
                    out=wpf, in_=proj_w.rearrange("(cb p) n -> p cb n", p=P))
                for ts in range(NQB):
                    for ocb in range(2):
                        py = psum.tile([P, 512], FP32, tag="mm", bufs=2)
                        for cb in range(NCB):
                            nc.tensor.matmul(
                                py, O_T[:, cb, ts * P:(ts + 1) * P],
                                wpf[:, cb, ocb * 512:(ocb + 1) * 512],
                                start=(cb == 0), stop=(cb == NCB - 1))
                        nc.vector.tensor_add(
                            out=y_tok[:, ts, ocb * 512:(ocb + 1) * 512],
                            in0=py,
                            in1=x_tok[:, ts, ocb * 512:(ocb + 1) * 512])

            # ------------- P5: LN2 -> ln2T (channel-major) -------------
            with tc.tile_pool(name="ln2t_pool", bufs=1) as p_ln2t:
                ln2T = p_ln2t.tile([P, NCB, NQ], FP32R)
                with tc.tile_pool(name="p5s", bufs=1) as p5s:
                    ycts = []
                    for ts in range(NQB):
                        yc_t = p5s.tile([P, C], FP32, tag=f"yc_t{ts}", bufs=1,
                                        name=f"yc_t{ts}")
                        mean, rstd = _layernorm_stats(nc, stats_p, y_tok[:, ts, :])
                        nc.vector.tensor_scalar(
                            out=yc_t, in0=y_tok[:, ts, :], scalar1=mean,
                            scalar2=rstd, op0=ALU.subtract, op1=ALU.mult,
                        )
                        ycts.append(yc_t)
                    # cb-outer so ln2T[:, cb, :] completes per channel block
                    # and fc1's accumulation can begin after the first one
                    for cb in range(NCB):
                        for ts in range(NQB):
                            pt = psum.tile([P, P], FP32, tag="misc", bufs=2)
                            nc.tensor.transpose(
                                pt, ycts[ts][:, cb * P:(cb + 1) * P], ident)
                            nc.scalar.activation(
                                out=ln2T[:, cb, ts * P:(ts + 1) * P], in_=pt,
                                func=AF.Identity, scale=g2[:, cb:cb + 1],
                                bias=b2[:, cb:cb + 1],
                            )

                # ------------- P6: fc1 + GELU -> h1T -------------
                with tc.tile_pool(name="h1_pool", bufs=1) as p_h1:
                    h1T = p_h1.tile([P, NHB, NQ], FP32R)
                    with tc.tile_pool(name="p6s", bufs=1) as p6s:
                        for hc in range(4):  # 8-hb chunks of fc1_w
                            w1 = p6s.tile([P, NCB, 8 * P], FP32R, tag="w1",
                                          bufs=2)
                            nc.scalar.dma_start(
                                out=w1,
                                in_=fc1_w[:, hc * 8 * P:(hc + 1) * 8 * P]
                                .rearrange("(cb p) n -> p cb n", p=P),
                            )
                            for hl in range(8):
                                hb = hc * 8 + hl
                                ph = psum.tile([P, 512], FP32, tag="mm", bufs=2)
                                for cb in range(NCB):
                                    nc.tensor.matmul(
                                        ph, w1[:, cb, hl * P:(hl + 1) * P],
                                        ln2T[:, cb, :],
                                        start=(cb == 0), stop=(cb == NCB - 1))
                                nc.scalar.activation(
                                    out=h1T[:, hb, :], in_=ph, func=AF.Gelu,
                                    bias=f1b[:, hb:hb + 1], scale=1.0)

                    # ------------- P7: fc2 + residual -> out -------------
                    # swapped operands: lhsT = h1T (hidden-major), rhs =
                    # natural fc2_w rows -> token-major out, no transposes.
                    # 8 psum accumulators (4 ts x 2 ocb) live across the 4
                    # hb-chunks, overlapping fc1 production order.
                    with tc.tile_pool(name="p7s", bufs=1) as p7s:
                        ob_bc = bass.AP(tensor=fc2_b.tensor, offset=fc2_b.offset,
                                        ap=[[0, P], [1, C]])
                        obt = p7s.tile([P, C], FP32, tag="obt", bufs=1)
                        nc.sync.dma_start(out=obt, in_=ob_bc)
                        for ts in range(NQB):
                            nc.vector.tensor_add(out=y_tok[:, ts, :],
                                                 in0=y_tok[:, ts, :], in1=obt)
                        out_tok = p7s.tile([P, NQB, C], FP32, tag="out_tok",
                                           bufs=1)
                        pos = [psum.tile([P, 2, 512], FP32, tag="mm", bufs=2,
                                         name=f"po_mm{i}") for i in range(2)]
                        poa = [psum.tile([P, 512], FP32, tag="acc", bufs=2,
                                         name=f"po_acc{i}") for i in range(2)]
                        pom = [psum.tile([P, 512], FP32, tag="misc", bufs=2,
                                         name=f"po_misc{i}") for i in range(2)]
                        po = {(0, 0): pos[0][:, 0, :], (0, 1): pos[0][:, 1, :],
                              (1, 0): pos[1][:, 0, :], (1, 1): pos[1][:, 1, :],
                              (2, 0): poa[0], (2, 1): poa[1],
                              (3, 0): pom[0], (3, 1): pom[1]}
                        for hc in range(4):
                            w2 = p7s.tile([P, 8, C], FP32R, tag="w2", bufs=2)
                            nc.scalar.dma_start(
                                out=w2,
                                in_=fc2_w[hc * 8 * P:(hc + 1) * 8 * P, :]
                                .rearrange("(hb p) n -> p hb n", p=P),
                            )
                            for hl in range(8):
                                hb = hc * 8 + hl
                                for ts in range(NQB):
                                    for ocb in range(2):
                                        nc.tensor.matmul(
                                            po[(ts, ocb)],
                                            h1T[:, hb, ts * P:(ts + 1) * P],
                                            w2[:, hl, ocb * 512:(ocb + 1) * 512],
                                            start=(hb == 0), stop=(hb == NHB - 1))
                        for ts in range(NQB):
                            for ocb in range(2):
                                nc.vector.tensor_add(
                                    out=out_tok[:, ts, ocb * 512:(ocb + 1) * 512],
                                    in0=po[(ts, ocb)],
                                    in1=y_tok[:, ts, ocb * 512:(ocb + 1) * 512])
                        nc.sync.dma_start(
                            out=out.rearrange("(tb p) c -> p tb c", p=P),
                            in_=out_tok)

    _split_waits(nc)
    return nc


_NC_CACHE = None


def kernel(**inputs):
    global _NC_CACHE
    if _NC_CACHE is None:
        _NC_CACHE = build_program()
    nc = _NC_CACHE

    x = np.ascontiguousarray(np.asarray(inputs["x"], dtype=np.float32))
    weights = {
        k: np.ascontiguousarray(np.asarray(inputs[k], dtype=np.float32))
        for k in ("ln1_g", "ln1_b", "qkv_w", "proj_w", "proj_b",
                  "ln2_g", "ln2_b", "fc1_w", "fc1_b", "fc2_w", "fc2_b")
    }
    in_maps = []
    for c in range(NCORES):
        b, q0 = c // 4, NQ * (c % 4)
        xb = np.ascontiguousarray(np.roll(x[b], -q0, axis=0))
        in_maps.append({"x": xb, **weights})

    res = run_bass_kernel_spmd(nc, in_maps, list(range(NCORES)))
    out = np.empty((B, N, C), dtype=np.float32)
    for c in range(NCORES):
        b, q0 = c // 4, NQ * (c % 4)
        out[b, q0:q0 + NQ] = res.results[c]["out"]
    return out


# revision 30
# speedup vs baseline: 1.1241x; 1.0516x over previous
"""Trainium2 Bass kernel for a pre-norm transformer block (dense_transformer).

Full (unsharded) contract: kernel(**inputs) takes the tensors from
reference.setup_inputs() and returns the full [2, 2048, 1024] output.

Sharding: 8 cores; core c owns batch element b = c//4 and the 512-token
query slice q0 = 512*(c%4) of that batch element.  The host rolls each
core's copy of x[b] by -q0 so that every core's query tokens are rows
0:512 of its input — attention is invariant to key permutation, so K/V
computed from the rolled sequence are exact.  No cross-core collectives:
each core redundantly computes LN1 + K/V for its full batch element
(4 cores share a batch element), then Q/attention/proj/MLP only for its
own 512 tokens.

Layouts on-core (P = 128 partitions):
  ln1T  [128, 8, 2048]  channel-major LN1 output (C on partitions)
  K^T   [128, 2048]     per head-pair (2 heads x 64 dh on partitions)
  Q^T   [128, 512]      per head-pair
  V_g   [128, 16, 520]  token-major V for 8 heads, 65-wide per-head slots
                        with a ones column fused in (col 64) so the AV
                        matmul also yields the softmax denominator
  scores^T [128k, 512q] psum per k-block, exp'd on ScalarE, then
  o~    [65, 512]       psum accumulator over 16 k-blocks (row 64 = l)
  O^T   [128, 8, 512]   normalized attention output, channel-major
  y_tok [128, 4, 1024]  token-major residual stream (after proj)
  ln2T  [128, 8, 512]   channel-major LN2 output
  h1T   [128, 32, 512]  hidden-major GELU(fc1) output
Dense matmuls run as float32r (~fp32 accuracy at full PE rate for free
dim 512); the attention K^T/Q^T/V tiles and exp outputs are bf16.
"""

import sys

for _p in ("/root/.axon_site/_ro/trn_rl_repo", "/opt/trn_rl_repo"):
    if _p not in sys.path:
        sys.path.append(_p)

import numpy as np

import bass_rust
import concourse.bass as bass
import concourse.mybir as mybir
import concourse.tile as tile
from concourse.bass_utils import run_bass_kernel_spmd
from concourse.masks import make_identity
from concourse.vector_clock import ScopedClock

B, N, C = 2, 2048, 1024
H, DH = 16, 64
FF = 4096
NCORES = 8
NQ = 512          # query tokens per core
P = 128
EPS = 1e-5
SCALE = DH ** -0.5
FP32 = mybir.dt.float32
FP32R = mybir.dt.float32r
BF16 = mybir.dt.bfloat16
AF = mybir.ActivationFunctionType
ALU = mybir.AluOpType

NTB = N // P      # 16 token blocks of the full sequence
NCB = C // P      # 8 channel blocks
NQB = NQ // P     # 4 query token blocks
NHB = FF // P     # 32 hidden blocks
SLOT = DH + 1     # 65: V columns per head incl. the fused ones column


class SplitDrainTileContext(tile.TileContext):
    """TileContext whose tail drain carries at most one sem wait per
    instruction — this walrus build rejects >2 sync waits per instruction
    (CoreV3GenImpl setupSyncWait: "Too many sync wait commands")."""

    def _drain_and_barrier(self, tick_clock, wait_clock):
        nc = self.nc
        probe = nc.sync.nop(nofuse=True)
        wait_clock.add_sem_waits(
            probe.ins, ScopedClock({None: tick_clock.global_clock})
        )
        si = probe.ins.sync_info
        waits = list(si.on_wait) if si is not None else []
        updates = list(si.on_update) if si is not None else []
        probe.ins.sync_info = bass_rust.SyncInfo(on_wait=waits[:1], on_update=updates)
        for w in waits[1:]:
            extra = nc.sync.nop(nofuse=True)
            extra.ins.sync_info = bass_rust.SyncInfo(on_wait=[w], on_update=[])
        # Body of TileContext._drain_and_barrier minus add_sem_waits (the
        # waits now live on the nop chain above).
        nc.sync.drain()
        nc.all_engine_barrier()
        assert self.sems is not None
        popped = nc._tile_sem_poison_stack.pop()
        assert popped is self._sem_poison
        nc.clear_and_free_semaphores(list(self.sems.allocated().values()))
        nc.all_engine_barrier()


def _split_waits(nc, maxw=1):
    """Hoist excess sync waits onto same-engine NOPs: this walrus build
    rejects instructions carrying more than `maxw` sync wait commands."""
    snapshots = []
    for f in nc.m.functions:
        for blk in f.blocks:
            snapshots.append((blk, list(blk.instructions)))
    for blk, insts in snapshots:
        rebuilt = []
        for inst in insts:
            si = inst.sync_info
            waits = list(si.on_wait) if si is not None else []
            if len(waits) > maxw:
                for w in waits[:-maxw]:
                    nop = nc.engines[inst.engine].nop(nofuse=True).ins
                    nop.sync_info = bass_rust.SyncInfo(on_wait=[w], on_update=[])
                    rebuilt.append(nop)
                inst.sync_info = bass_rust.SyncInfo(
                    on_wait=waits[-maxw:], on_update=list(si.on_update))
            rebuilt.append(inst)
        blk.instructions = rebuilt


def _layernorm_stats(nc, pool, xt):
    """mean/rstd of xt [128, 1024] over the free axis -> ([128,1], [128,1])."""
    sub = xt.rearrange("p (s f) -> p s f", f=512)
    stats = pool.tile([P, 2, 6], FP32, tag="ln_stats", bufs=4)
    for s in range(2):
        nc.vector.bn_stats(out=stats[:, s, :], in_=sub[:, s, :])
    mv = pool.tile([P, 2], FP32, tag="ln_mv", bufs=4)
    nc.vector.bn_aggr(out=mv[:], in_=stats[:])
    eps = pool.tile([P, 1], FP32, tag="ln_eps", bufs=1)
    nc.vector.memset(eps, EPS)
    rstd = pool.tile([P, 1], FP32, tag="ln_rstd", bufs=4)
    nc.scalar.activation(out=rstd, in_=mv[:, 1:2], func=AF.Sqrt, bias=eps, scale=1.0)
    nc.vector.reciprocal(out=rstd, in_=rstd)
    return mv[:, 0:1], rstd


def build_program():
    nc = bass.Bass("TRN2", target_bir_lowering=False, debug=False)

    x = nc.declare_dram_parameter("x", [N, C], FP32, isOutput=False).ap()
    ln1_g = nc.declare_dram_parameter("ln1_g", [C], FP32, isOutput=False).ap()
    ln1_b = nc.declare_dram_parameter("ln1_b", [C], FP32, isOutput=False).ap()
    qkv_w = nc.declare_dram_parameter("qkv_w", [C, 3 * C], FP32R, isOutput=False).ap()
    proj_w = nc.declare_dram_parameter("proj_w", [C, C], FP32R, isOutput=False).ap()
    proj_b = nc.declare_dram_parameter("proj_b", [C], FP32, isOutput=False).ap()
    ln2_g = nc.declare_dram_parameter("ln2_g", [C], FP32, isOutput=False).ap()
    ln2_b = nc.declare_dram_parameter("ln2_b", [C], FP32, isOutput=False).ap()
    fc1_w = nc.declare_dram_parameter("fc1_w", [C, FF], FP32R, isOutput=False).ap()
    fc1_b = nc.declare_dram_parameter("fc1_b", [FF], FP32, isOutput=False).ap()
    fc2_w = nc.declare_dram_parameter("fc2_w", [FF, C], FP32R, isOutput=False).ap()
    fc2_b = nc.declare_dram_parameter("fc2_b", [C], FP32, isOutput=False).ap()
    out = nc.declare_dram_parameter("out", [NQ, C], FP32, isOutput=True).ap()

    x_t = x.rearrange("(tb p) c -> p tb c", p=P)

    with SplitDrainTileContext(nc) as tc:
        with (
            tc.tile_pool(name="consts", bufs=1) as consts,
            tc.tile_pool(name="stats", bufs=1) as stats_p,
            tc.tile_pool(name="y_pool", bufs=1) as y_pool,
            tc.tile_pool(name="ot_pool", bufs=1) as ot_pool,
            tc.tile_pool(name="psum", bufs=1, space="PSUM") as psum,
        ):
            ident = consts.tile([P, P], FP32)
            make_identity(nc, ident)
            ones32 = consts.tile([P, NTB, 8], FP32)
            nc.vector.memset(ones32, 1.0)
            ones_f = consts.tile([P, DH], FP32)
            nc.vector.memset(ones_f, 1.0)
            ones_col = consts.tile([P, DH], FP32R)
            nc.vector.tensor_copy(out=ones_col, in_=ones_f)

            # per-channel vectors in channel-major [128, NCB] layout
            g1 = consts.tile([P, NCB], FP32)
            b1 = consts.tile([P, NCB], FP32)
            g2 = consts.tile([P, NCB], FP32)
            b2 = consts.tile([P, NCB], FP32)
            pb = consts.tile([P, NCB], FP32)
            fc2b = consts.tile([P, NCB], FP32)
            f1b = consts.tile([P, NHB], FP32)
            for dst, src in ((g1, ln1_g), (b1, ln1_b), (g2, ln2_g), (b2, ln2_b),
                             (pb, proj_b), (fc2b, fc2_b)):
                nc.sync.dma_start(out=dst, in_=src.rearrange("(cb p) -> p cb", p=P))
            nc.sync.dma_start(out=f1b, in_=fc1_b.rearrange("(hb p) -> p hb", p=P))

            y_tok = y_pool.tile([P, NQB, C], FP32)
            O_T = ot_pool.tile([P, NCB, NQ], FP32R)

            # ------------- P0: LN1 + transpose to channel-major -------------
            with tc.tile_pool(name="ln1t_pool", bufs=1) as p_ln1t:
                ln1T = p_ln1t.tile([P, NCB, N], FP32R)
                with tc.tile_pool(name="p0s", bufs=1) as p0s:
                    for tbg in range(NTB // 4):
                        xts = []
                        for i in range(4):
                            tb = 4 * tbg + i
                            xt = p0s.tile([P, C], FP32, tag=f"x_in{i}", bufs=2,
                                          name=f"xt{i}")
                            nc.sync.dma_start(out=xt, in_=x_t[:, tb, :])
                            mean, rstd = _layernorm_stats(nc, stats_p, xt)
                            nc.vector.tensor_scalar(
                                out=xt, in0=xt, scalar1=mean, scalar2=rstd,
                                op0=ALU.subtract, op1=ALU.mult,
                            )
                            xts.append(xt)
                        # 4 transposes share one PSUM bank; a single ACT op
                        # evacuates 512 contiguous ln1T columns per (cb, tbg)
                        for cb in range(NCB):
                            pt = psum.tile([P, 512], FP32, tag="misc", bufs=2)
                            for i in range(4):
                                nc.tensor.transpose(
                                    pt[:, i * P:(i + 1) * P],
                                    xts[i][:, cb * P:(cb + 1) * P], ident)
                            nc.scalar.activation(
                                out=ln1T[:, cb, tbg * 512:(tbg + 1) * 512],
                                in_=pt, func=AF.Identity,
                                scale=g1[:, cb:cb + 1], bias=b1[:, cb:cb + 1],
                            )

                # ------------- P1-P3: QKV projections + attention -------------
                with tc.tile_pool(name="p1s", bufs=1) as p1s:
                    pending = None

                    def emit_normalize(pair, o_rawA, o_rawB, rl):
                        bca = psum.tile([P, 512], FP32, tag="misc", bufs=2,
                                        name="bca")
                        nc.tensor.matmul(
                            bca[0:DH, :], ones_col[DH:DH + 1, :],
                            rl[DH:DH + 1, 0:512])
                        nc.vector.tensor_mul(out=O_T[0:DH, pair, :],
                                             in0=o_rawA[0:DH, :],
                                             in1=bca[0:DH, :])
                        bcb = psum.tile([P, 512], FP32, tag="misc", bufs=2,
                                        name="bcb")
                        nc.tensor.matmul(
                            bcb[0:DH, :], ones_col[DH:DH + 1, :],
                            rl[DH:DH + 1, 512:1024])
                        # odd head lands on partitions 64:128 of O_T; DVE ops
                        # are partition-aligned, so normalize at base 0 and
                        # move via SBUF->SBUF DMA (partition crossbar)
                        o_sb = p1s.tile([DH, 512], FP32R, tag="o_sb", bufs=2,
                                        name="o_sb")
                        nc.vector.tensor_mul(out=o_sb, in0=o_rawB[0:DH, :],
                                             in1=bcb[0:DH, :])
                        nc.sync.dma_start(out=O_T[DH:P, pair, :], in_=o_sb)

                    for g in range(2):  # head groups of 8 heads
                        V_g = p1s.tile([P, NTB, 8 * SLOT], BF16, tag="V_g", bufs=1)
                        v4 = V_g.rearrange("p t (h s) -> p t h s", s=SLOT)
                        # memset can't produce fp32r on this ISA; cast-copy ones
                        nc.vector.tensor_copy(out=v4[:, :, :, DH:DH + 1],
                                              in_=ones32[:, :, :, None])

                        wv = p1s.tile([P, NCB, 512], FP32R, tag="wv", bufs=1)
                        nc.scalar.dma_start(
                            out=wv,
                            in_=qkv_w[:, 2 * C + 512 * g: 2 * C + 512 * (g + 1)]
                            .rearrange("(cb p) n -> p cb n", p=P),
                        )
                        for tb in range(NTB):
                            pv = psum.tile([P, 512], FP32, tag="mm", bufs=2)
                            for cb in range(NCB):
                                nc.tensor.matmul(
                                    pv, ln1T[:, cb, tb * P:(tb + 1) * P],
                                    wv[:, cb, :],
                                    start=(cb == 0), stop=(cb == NCB - 1),
                                )
                            pvh = pv.rearrange("p (h s) -> p h s", s=DH)
                            nc.vector.tensor_copy(out=v4[:, tb, :, 0:DH], in_=pvh)

                        wkg = p1s.tile([P, NCB, 512], FP32R, tag="wkg", bufs=1)
                        nc.scalar.dma_start(
                            out=wkg,
                            in_=qkv_w[:, C + 512 * g: C + 512 * (g + 1)]
                            .rearrange("(cb p) n -> p cb n", p=P),
                        )
                        wqg = p1s.tile([P, NCB, 512], FP32R, tag="wqg", bufs=1)
                        nc.scalar.dma_start(
                            out=wqg,
                            in_=qkv_w[:, 512 * g: 512 * (g + 1)]
                            .rearrange("(cb p) n -> p cb n", p=P),
                        )
                        for pr in range(4):  # head pairs within the group
                            pair = 4 * g + pr
                            wk = wkg[:, :, pr * P:(pr + 1) * P]
                            wq = wqg[:, :, pr * P:(pr + 1) * P]
                            KT = p1s.tile([P, N], BF16, tag="KT", bufs=2)
                            for tb in range(4):  # 512-token blocks
                                pk = psum.tile([P, 512], FP32, tag="mm", bufs=2)
                                for cb in range(NCB):
                                    nc.tensor.matmul(
                                        pk, wk[:, cb, :],
                                        ln1T[:, cb, tb * 512:(tb + 1) * 512],
                                        start=(cb == 0), stop=(cb == NCB - 1),
                                    )
                                nc.vector.tensor_copy(
                                    out=KT[:, tb * 512:(tb + 1) * 512], in_=pk)
                            QT = p1s.tile([P, NQ], BF16, tag="QT", bufs=2)
                            pq = psum.tile([P, 512], FP32, tag="mm", bufs=2)
                            for cb in range(NCB):
                                nc.tensor.matmul(
                                    pq, wq[:, cb, :], ln1T[:, cb, 0:NQ],
                                    start=(cb == 0), stop=(cb == NCB - 1),
                                )
                            nc.vector.tensor_copy(out=QT, in_=pq)

                            # flash attention over 16 key blocks; the two heads
                            # of the pair run as packed K=64 row-tiles
                            oa = psum.tile([P, 512], FP32, tag="acc", bufs=2)
                            ob_ = psum.tile([P, 512], FP32, tag="acc", bufs=2)
                            sl_a = slice(2 * pr * SLOT, (2 * pr) * SLOT + SLOT)
                            sl_b = slice((2 * pr + 1) * SLOT, (2 * pr + 2) * SLOT)
                            def emit_av(k2, ea, eb):
                                for j in range(2):
                                    kb = 2 * k2 + j
                                    nc.tensor.matmul(
                                        oa[0:SLOT, :], V_g[:, kb, sl_a],
                                        ea[:, j, :],
                                        start=(kb == 0), stop=(kb == NTB - 1),
                                    )
                                    nc.tensor.matmul(
                                        ob_[0:SLOT, :], V_g[:, kb, sl_b],
                                        eb[:, j, :],
                                        start=(kb == 0), stop=(kb == NTB - 1),
                                    )

                            av_pending = None
                            for k2 in range(NTB // 2):
                                sa = psum.tile([P, 2, 512], FP32, tag="mm", bufs=2)
                                sb = psum.tile([P, 2, 512], FP32, tag="mm", bufs=2)
                                for j in range(2):
                                    kb = 2 * k2 + j
                                    ks = slice(kb * P, (kb + 1) * P)
                                    nc.tensor.matmul(
                                        sa[:, j, :], KT[0:DH, ks], QT[0:DH, :],
                                        tile_position=(0, 0),
                                    )
                                    nc.tensor.matmul(
                                        sb[:, j, :], KT[DH:P, ks], QT[DH:P, :],
                                        tile_position=(DH, 0),
                                    )
                                ea = p1s.tile([P, 2, 512], BF16, tag="ea", bufs=3)
                                nc.scalar.activation(out=ea, in_=sa, func=AF.Exp,
                                                     scale=SCALE)
                                eb = p1s.tile([P, 2, 512], BF16, tag="eb", bufs=3)
                                nc.scalar.activation(out=eb, in_=sb, func=AF.Exp,
                                                     scale=SCALE)
                                if av_pending is not None:
                                    emit_av(*av_pending)
                                av_pending = (k2, ea, eb)
                            emit_av(*av_pending)
                            # evacuate o~ + l to SBUF right away (frees the
                            # PSUM accumulators for the next pair), 1/l on DVE,
                            # and defer broadcast+scale one pair so the PE
                            # never stalls on the reciprocal
                            o_rawA = p1s.tile([SLOT, 512], FP32, tag="o_rawA",
                                              bufs=2)
                            nc.vector.tensor_copy(out=o_rawA, in_=oa[0:SLOT, :])
                            o_rawB = p1s.tile([SLOT, 512], FP32, tag="o_rawB",
                                              bufs=2)
                            nc.vector.tensor_copy(out=o_rawB, in_=ob_[0:SLOT, :])
                            rl = stats_p.tile([P, 2 * 512], FP32R, tag="rl",
                                              bufs=2)
                            with nc.allow_low_precision(
                                    reason="softmax 1/l feeds an fp32r matmul"):
                                nc.vector.reciprocal(out=rl[DH:DH + 1, 0:512],
                                                     in_=o_rawA[DH:DH + 1, :])
                                nc.vector.reciprocal(out=rl[DH:DH + 1, 512:1024],
                                                     in_=o_rawB[DH:DH + 1, :])
                            if pending is not None:
                                emit_normalize(*pending)
                            pending = (pair, o_rawA, o_rawB, rl)

                    if pending is not None:
                        emit_normalize(*pending)
                        pending = None

            # ------------- P4: proj + residual -> y_tok -------------
            # swapped operands: lhsT = O^T (channel-major), rhs = natural
            # proj_w rows -> psum is token-major y directly (no transposes)
            with tc.tile_pool(name="p4s", bufs=1) as p4s:
                x_tok = p4s.tile([P, NQB, C], FP32, tag="x_res", bufs=1)
                nc.sync.dma_start(out=x_tok, in_=x_t[:, 0:NQB, :])
                pb_bc = bass.AP(tensor=proj_b.tensor, offset=proj_b.offset,
                                ap=[[0, P], [1, C]])
                pbt = p4s.tile([P, C], FP32, tag="pbt", bufs=1)
                nc.sync.dma_start(out=pbt, in_=pb_bc)
                for ts in range(NQB):
                    nc.vector.tensor_add(out=x_tok[:, ts, :],
                                         in0=x_tok[:, ts, :], in1=pbt)
                wpf = p4s.tile([P, NCB, C], FP32R, tag="wpf", bufs=1)
                nc.scalar.dma_start(
                    out=wpf, in_=proj_w.rearrange("(cb p) n -> p cb n", p=P))
                for ts in range(NQB):
                    for ocb in range(2):
                        py = psum.tile([P, 512], FP32, tag="mm", bufs=2)
                        for cb in range(NCB):
                            nc.tensor.matmul(
                                py, O_T[:, cb, ts * P:(ts + 1) * P],
                                wpf[:, cb, ocb * 512:(ocb + 1) * 512],
                                start=(cb == 0), stop=(cb == NCB - 1))
                        nc.vector.tensor_add(
                            out=y_tok[:, ts, ocb * 512:(ocb + 1) * 512],
                            in0=py,
                            in1=x_tok[:, ts, ocb * 512:(ocb + 1) * 512])

            # ------------- P5: LN2 -> ln2T (channel-major) -------------
            with tc.tile_pool(name="ln2t_pool", bufs=1) as p_ln2t:
                ln2T = p_ln2t.tile([P, NCB, NQ], FP32R)
                with tc.tile_pool(name="p5s", bufs=1) as p5s:
                    ycts = []
                    for ts in range(NQB):
                        yc_t = p5s.tile([P, C], FP32, tag=f"yc_t{ts}", bufs=1,
                                        name=f"yc_t{ts}")
                        mean, rstd = _layernorm_stats(nc, stats_p, y_tok[:, ts, :])
                        nc.vector.tensor_scalar(
                            out=yc_t, in0=y_tok[:, ts, :], scalar1=mean,
                            scalar2=rstd, op0=ALU.subtract, op1=ALU.mult,
                        )
                        ycts.append(yc_t)
                    # cb-outer so ln2T[:, cb, :] completes per channel block
                    # and fc1's accumulation can begin after the first one
                    for cb in range(NCB):
                        pt = psum.tile([P, 512], FP32, tag="misc", bufs=2)
                        for ts in range(NQB):
                            nc.tensor.transpose(
                                pt[:, ts * P:(ts + 1) * P],
                                ycts[ts][:, cb * P:(cb + 1) * P], ident)
                        nc.scalar.activation(
                            out=ln2T[:, cb, :], in_=pt, func=AF.Identity,
                            scale=g2[:, cb:cb + 1], bias=b2[:, cb:cb + 1],
                        )

                # ------------- P6: fc1 + GELU -> h1T -------------
                with tc.tile_pool(name="h1_pool", bufs=1) as p_h1:
                    h1T = p_h1.tile([P, NHB, NQ], FP32R)
                    with tc.tile_pool(name="p6s", bufs=1) as p6s:
                        for hc in range(4):  # 8-hb chunks of fc1_w
                            w1 = p6s.tile([P, NCB, 8 * P], FP32R, tag="w1",
                                          bufs=2)
                            nc.scalar.dma_start(
                                out=w1,
                                in_=fc1_w[:, hc * 8 * P:(hc + 1) * 8 * P]
                                .rearrange("(cb p) n -> p cb n", p=P),
                            )
                            for hl in range(8):
                                hb = hc * 8 + hl
                                ph = psum.tile([P, 512], FP32, tag="mm", bufs=2)
                                for cb in range(NCB):
                                    nc.tensor.matmul(
                                        ph, w1[:, cb, hl * P:(hl + 1) * P],
                                        ln2T[:, cb, :],
                                        start=(cb == 0), stop=(cb == NCB - 1))
                                nc.scalar.activation(
                                    out=h1T[:, hb, :], in_=ph, func=AF.Gelu,
                                    bias=f1b[:, hb:hb + 1], scale=1.0)

                    # ------------- P7: fc2 + residual -> out -------------
                    # swapped operands: lhsT = h1T (hidden-major), rhs =
                    # natural fc2_w rows -> token-major out, no transposes.
                    # 8 psum accumulators (4 ts x 2 ocb) live across the 4
                    # hb-chunks, overlapping fc1 production order.
                    with tc.tile_pool(name="p7s", bufs=1) as p7s:
                        ob_bc = bass.AP(tensor=fc2_b.tensor, offset=fc2_b.offset,
                                        ap=[[0, P], [1, C]])
                        obt = p7s.tile([P, C], FP32, tag="obt", bufs=1)
                        nc.sync.dma_start(out=obt, in_=ob_bc)
                        for ts in range(NQB):
                            nc.vector.tensor_add(out=y_tok[:, ts, :],
                                                 in0=y_tok[:, ts, :], in1=obt)
                        out_tok = p7s.tile([P, NQB, C], FP32, tag="out_tok",
                                           bufs=1)
                        pos = [psum.tile([P, 2, 512], FP32, tag="mm", bufs=2,
                                         name=f"po_mm{i}") for i in range(2)]
                        poa = [psum.tile([P, 512], FP32, tag="acc", bufs=2,
                                         name=f"po_acc{i}") for i in range(2)]
                        pom = [psum.tile([P, 512], FP32, tag="misc", bufs=2,
                                         name=f"po_misc{i}") for i in range(2)]
                        po = {(0, 0): pos[0][:, 0, :], (0, 1): pos[0][:, 1, :],
                              (1, 0): pos[1][:, 0, :], (1, 1): pos[1][:, 1, :],
                              (2, 0): poa[0], (2, 1): poa[1],
                              (3, 0): pom[0], (3, 1): pom[1]}
                        for hc in range(4):
                            w2 = p7s.tile([P, 8, C], FP32R, tag="w2", bufs=2)
                            nc.scalar.dma_start(
                                out=w2,
                                in_=fc2_w[hc * 8 * P:(hc + 1) * 8 * P, :]
                                .rearrange("(hb p) n -> p hb n", p=P),
                            )
                            for hl in range(8):
                                hb = hc * 8 + hl
                                for ts in range(NQB):
                                    for ocb in range(2):
                                        nc.tensor.matmul(
                                            po[(ts, ocb)],
                                            h1T[:, hb, ts * P:(ts + 1) * P],
                                            w2[:, hl, ocb * 512:(ocb + 1) * 512],
                                            start=(hb == 0), stop=(hb == NHB - 1))
                        for ts in range(NQB):
                            for ocb in range(2):
                                nc.vector.tensor_add(
                                    out=out_tok[:, ts, ocb * 512:(ocb + 1) * 512],
                                    in0=po[(ts, ocb)],
                                    in1=y_tok[:, ts, ocb * 512:(ocb + 1) * 512])
                        nc.sync.dma_start(
                            out=out.rearrange("(tb p) c -> p tb c", p=P),
                            in_=out_tok)

    _split_waits(nc)
    return nc


_NC_CACHE = None


def kernel(**inputs):
    global _NC_CACHE
    if _NC_CACHE is None:
        _NC_CACHE = build_program()
    nc = _NC_CACHE

    x = np.ascontiguousarray(np.asarray(inputs["x"], dtype=np.float32))
    weights = {
        k: np.ascontiguousarray(np.asarray(inputs[k], dtype=np.float32))
        for k in ("ln1_g", "ln1_b", "qkv_w", "proj_w", "proj_b",
                  "ln2_g", "ln2_b", "fc1_w", "fc1_b", "fc2_w", "fc2_b")
    }
    in_maps = []
    for c in range(NCORES):
        b, q0 = c // 4, NQ * (c % 4)
        xb = np.ascontiguousarray(np.roll(x[b], -q0, axis=0))
        in_maps.append({"x": xb, **weights})

    res = run_bass_kernel_spmd(nc, in_maps, list(range(NCORES)))
    out = np.empty((B, N, C), dtype=np.float32)
    for c in range(NCORES):
        b, q0 = c // 4, NQ * (c % 4)
        out[b, q0:q0 + NQ] = res.results[c]["out"]
    return out
